# revision 1
# baseline (speedup 1.0000x reference)
"""v6: fused phases — attention for a head pair starts right after its
qT/kT tiles are ready, overlapping ScalarE exp with TensorE qkv matmuls.
Scores for the two heads of a pair are row-tiled (partitions 0:64 / 64:128)
into one 2-bank psum tile so they run concurrently on the PE and share one
big exp instruction.

x is streamed 3x (once per head pair); V is computed during the first pass.
cos/sin shipped as fp16 to save SBUF.
"""

import os
import sys

for _p in ("/opt/trn_rl_repo", "/root/.axon_site/_ro/trn_rl_repo"):
    if os.path.isdir(_p) and _p not in sys.path:
        sys.path.insert(0, _p)

import contextlib

import numpy as np

import concourse.bass as bass
import concourse.tile as tile
from concourse import bacc, mybir
from concourse.bass_utils import run_bass_kernel_spmd

P = 128
L = 2048
D = 1536
HL = 6
HD = 64
EQ = 384
NQK = 768
DC = D // P      # 12
LT = L // P      # 16
ACH = 512        # attention lq chunk
XCH = 256        # qkv l chunk
F32 = mybir.dt.float32
F32R = mybir.dt.float32r
F16 = mybir.dt.float16
AF = mybir.ActivationFunctionType


def build_bass(repeat=1):
    nc = bacc.Bacc("TRN2", target_bir_lowering=False, debug=False, num_devices=8)
    xT = nc.dram_tensor("xT", [D, L], F32R, kind="ExternalInput")
    wqkT = nc.dram_tensor("wqkT", [D, NQK], F32R, kind="ExternalInput")
    wvT = nc.dram_tensor("wvT", [D, EQ], F32R, kind="ExternalInput")
    woT = nc.dram_tensor("woT", [EQ, D], F32R, kind="ExternalInput")
    cos2 = nc.dram_tensor("cos2", [P, L], F16, kind="ExternalInput")
    ss2 = nc.dram_tensor("ss2", [P, L], F16, kind="ExternalInput")
    out = nc.dram_tensor("out", [L, D], F32, kind="ExternalOutput")

    xT_r = xT.rearrange("(dc p) l -> p dc l", p=P)
    wqkT_r = wqkT.rearrange("(dc p) e -> p dc e", p=P)
    wvT_r = wvT.rearrange("(dc p) e -> p dc e", p=P)
    woT_r = woT.rearrange("(ec p) d -> p ec d", p=P)

    with tile.TileContext(nc) as tc:
        rep_cm = tc.For_i(0, repeat, 1) if repeat > 1 else contextlib.nullcontext()
        with rep_cm, tc.tile_pool(name="persist", bufs=1) as persist:
            qT = persist.tile([P, 3, L], F32R)
            kT = persist.tile([P, 3, L], F32R)
            v1 = persist.tile([P, LT, HL, HD + 1], F32R)
            cos_sb = persist.tile([P, L], F16)
            ss_sb = persist.tile([P, L], F16)
            outT = persist.tile([P, 3, L], F32R)
            ones_c = nc.const_aps.tensor(1.0, (P, 1), F32)
            nc.vector.tensor_copy(
                v1[:, :, :, HD : HD + 1], ones_c.to_broadcast([P, LT, HL, 1])
            )
            nc.sync.dma_start(cos_sb[:], cos2[:])
            nc.sync.dma_start(ss_sb[:], ss2[:])

            with (
                tc.tile_pool(name="s2w", bufs=2) as s2w,
                tc.tile_pool(name="s2x", bufs=2) as s2x,
                tc.tile_pool(name="s2t", bufs=2) as s2t,
                tc.tile_pool(name="s2att", bufs=2) as s2att,
                tc.tile_pool(name="s2o", bufs=3) as s2o,
                tc.tile_pool(name="s2nrm", bufs=3) as s2nrm,
                tc.tile_pool(name="ps_acc", bufs=2, space=bass.MemorySpace.PSUM) as ps_acc,
                tc.tile_pool(name="ps_s", bufs=2, space=bass.MemorySpace.PSUM) as ps_s,
                tc.tile_pool(name="ps_av", bufs=2, space=bass.MemorySpace.PSUM) as ps_av,
            ):
                for etp in range(3):
                    # --- load this pair's qk weights (and v weights at etp 0)
                    wqks = s2w.tile([P, DC, EQ], F32R, tag="w")
                    for d0 in range(0, DC, 3):
                        dsl = slice(d0, d0 + 3)
                        nc.sync.dma_start(
                            wqks[:, dsl, 0:P], wqkT_r[:, dsl, etp * P : (etp + 1) * P]
                        )
                        nc.sync.dma_start(
                            wqks[:, dsl, P : 2 * P],
                            wqkT_r[:, dsl, EQ + etp * P : EQ + (etp + 1) * P],
                        )
                    if etp == 0:
                        wv_sb = s2w.tile([P, DC, EQ], F32R, tag="w")
                        for d0 in range(0, DC, 3):
                            nc.sync.dma_start(
                                wv_sb[:, d0 : d0 + 3, :], wvT_r[:, d0 : d0 + 3, :]
                            )
                    if etp == 2:
                        wo_sb = s2w.tile([P, 3, D], F32R, tag="w")
                        nc.sync.dma_start(wo_sb[:], woT_r[:])

                    # --- stream x; qk projection + rope (+ V on first pass)
                    for c in range(L // XCH):
                        sl = slice(c * XCH, (c + 1) * XCH)
                        xc = s2x.tile([P, DC, XCH], F32R)
                        for d0 in range(0, DC, 3):
                            nc.sync.dma_start(
                                xc[:, d0 : d0 + 3, :], xT_r[:, d0 : d0 + 3, sl]
                            )

                        for half in range(2):  # 0: q tile, 1: k tile
                            ps = ps_acc.tile([P, ACH], F32, tag="acc")
                            for dc in range(DC):
                                nc.tensor.matmul(
                                    ps[:, 0:XCH],
                                    wqks[:, dc, half * P : (half + 1) * P],
                                    xc[:, dc, :],
                                    start=(dc == 0),
                                    stop=(dc == DC - 1),
                                )
                            dst = (qT if half == 0 else kT)[:, etp, sl]
                            tcos = s2t.tile([P, XCH], F32, tag="tcos")
                            trot = s2t.tile([P, XCH], F32, tag="trot")
                            nc.vector.tensor_mul(tcos[:], ps[:, 0:XCH], cos_sb[:, sl])
                            for q_ in range(4):
                                s = (q_ ^ 1) * 32
                                d_ = q_ * 32
                                nc.vector.tensor_mul(
                                    trot[d_ : d_ + 32, :],
                                    ps[s : s + 32, 0:XCH],
                                    ss_sb[d_ : d_ + 32, sl],
                                )
                            nc.vector.tensor_add(dst, tcos[:], trot[:])

                        if etp == 0:
                            for lt2 in range(XCH // P):
                                lk = c * (XCH // P) + lt2
                                pv = ps_acc.tile([P, ACH], F32, tag="acc")
                                for dc in range(DC):
                                    nc.tensor.matmul(
                                        pv[:, 0:EQ],
                                        xc[:, dc, lt2 * P : (lt2 + 1) * P],
                                        wv_sb[:, dc, :],
                                        start=(dc == 0),
                                        stop=(dc == DC - 1),
                                    )
                                nc.scalar.copy(
                                    v1[:, lk, :, 0:HD],
                                    pv[:, 0:EQ].rearrange("p (h d) -> p h d", h=HL),
                                )

                    # --- attention for heads 2*etp (rows 0:64) and 2*etp+1
                    for cq in range(L // ACH):
                        cqs = slice(cq * ACH, (cq + 1) * ACH)
                        pav0 = ps_av.tile([HD + 1, ACH], F32, tag="av")
                        pav1 = ps_av.tile([HD + 1, ACH], F32, tag="av")
                        for lk in range(LT):
                            pscore = ps_s.tile([P, 2 * ACH], F32)
                            att = s2att.tile([P, 2 * ACH], F32R)
                            for hh in range(2):  # row-tiled pair, concurrent
                                po = hh * HD
                                nc.tensor.matmul(
                                    pscore[:, hh * ACH : (hh + 1) * ACH],
                                    kT[po : po + HD, etp, lk * P : (lk + 1) * P],
                                    qT[po : po + HD, etp, cqs],
                                    start=True,
                                    stop=True,
                                )
                            nc.scalar.activation(att[:], pscore[:], AF.Exp, scale=0.125)
                            for hh, pav in ((0, pav0), (1, pav1)):
                                nc.tensor.matmul(
                                    pav[:],
                                    v1[:, lk, 2 * etp + hh, :],
                                    att[:, hh * ACH : (hh + 1) * ACH],
                                    start=(lk == 0),
                                    stop=(lk == LT - 1),
                                )
                        for hh, pav in ((0, pav0), (1, pav1)):
                            po = hh * HD
                            dcp = s2nrm.tile([1, ACH], F32, tag="dcp")
                            nc.vector.tensor_copy(dcp[:], pav[HD : HD + 1, :])
                            rcp = s2nrm.tile([1, ACH], F32, tag="rcp")
                            nc.vector.reciprocal_approx_fast(out=rcp[:], in_=dcp[:])
                            rb = s2nrm.tile([HD, ACH], F32, tag="rb")
                            nc.gpsimd.partition_broadcast(rb[:], rcp[:], channels=HD)
                            nc.vector.tensor_mul(
                                outT[po : po + HD, etp, cqs], pav[0:HD, :], rb[:]
                            )

                        if etp == 2:
                            # o-projection for this lq chunk (all heads done)
                            for lt in range(ACH // P):
                                l0 = cq * ACH + lt * P
                                for dn in range(D // ACH):
                                    pso = ps_acc.tile([P, ACH], F32, tag="acc")
                                    for ec in range(3):
                                        nc.tensor.matmul(
                                            pso[:],
                                            outT[:, ec, l0 : l0 + P],
                                            wo_sb[:, ec, dn * ACH : (dn + 1) * ACH],
                                            start=(ec == 0),
                                            stop=(ec == 2),
                                        )
                                    ot = s2o.tile([P, ACH], F32)
                                    nc.scalar.copy(ot[:], pso[:])
                                    nc.sync.dma_start(
                                        out[l0 : l0 + P, dn * ACH : (dn + 1) * ACH],
                                        ot[:],
                                    )

    nc.compile()
    return nc


_NC_CACHE = None


def _get_nc():
    global _NC_CACHE
    if _NC_CACHE is None:
        _NC_CACHE = build_bass()
    return _NC_CACHE


def make_in_maps(x, w_qkv, w_o, cos, sin):
    x = np.asarray(x, dtype=np.float32)
    w_qkv = np.asarray(w_qkv, dtype=np.float32)
    w_o = np.asarray(w_o, dtype=np.float32)
    cos = np.asarray(cos, dtype=np.float32)
    sin = np.asarray(sin, dtype=np.float32)

    cosT = np.ascontiguousarray(cos.T)
    sinT = sin.T
    ss = np.concatenate([-sinT[0:32], sinT[32:64]], axis=0)
    cos2 = np.ascontiguousarray(np.tile(cosT, (2, 1))).astype(np.float16)
    ss2 = np.ascontiguousarray(np.tile(ss, (2, 1))).astype(np.float16)

    in_maps = []
    for c in range(8):
        b, g = c // 4, c % 4
        xTc = np.ascontiguousarray(x[b].T)
        wq = w_qkv[g * EQ : (g + 1) * EQ]
        wk = w_qkv[D + g * EQ : D + (g + 1) * EQ]
        wv = w_qkv[2 * D + g * EQ : 2 * D + (g + 1) * EQ]
        wqkTc = np.ascontiguousarray(np.concatenate([wq, wk], 0).T)
        wvTc = np.ascontiguousarray(wv.T)
        woTc = np.ascontiguousarray(w_o[:, g * EQ : (g + 1) * EQ].T)
        in_maps.append(
            {
                "xT": xTc,
                "wqkT": wqkTc,
                "wvT": wvTc,
                "woT": woTc,
                "cos2": cos2,
                "ss2": ss2,
            }
        )
    return in_maps


def kernel(x, w_qkv, w_o, cos, sin):
    nc = _get_nc()
    in_maps = make_in_maps(x, w_qkv, w_o, cos, sin)
    res = run_bass_kernel_spmd(nc, in_maps, core_ids=list(range(8)))
    outs = [res.results[c]["out"] for c in range(8)]
    full = np.stack(
        [
            outs[0] + outs[1] + outs[2] + outs[3],
            outs[4] + outs[5] + outs[6] + outs[7],
        ]
    ).astype(np.float32)
    return full



# revision 2
# speedup vs baseline: 1.1980x; 1.1980x over previous
"""v7: fp16 datapath + resident x.

Same fused structure as v6 (attention for a head pair overlaps the next
pair's qkv matmuls; score pairs row-tiled into one 2-bank psum tile),
but all matmul operands are fp16 instead of fp32r:
 - halves SBUF/PE operand traffic (power-throttle relief: the profile
   shows the core duty-limited to 81%/50% by the activity manager)
 - halves DMA bytes (inputs shipped as fp16 from the host)
 - x is loaded once into SBUF (48KB/partition at fp16) instead of being
   re-streamed from HBM for every head pair (48MB -> ~10MB HBM reads).
PSUM accumulation stays fp32; rope & softmax normalization math in fp32.
"""

import os
import sys

for _p in ("/opt/trn_rl_repo", "/root/.axon_site/_ro/trn_rl_repo"):
    if os.path.isdir(_p) and _p not in sys.path:
        sys.path.insert(0, _p)

import contextlib

import numpy as np

import concourse.bass as bass
import concourse.tile as tile
from concourse import bacc, mybir
from concourse.bass_utils import run_bass_kernel_spmd

P = 128
L = 2048
D = 1536
HL = 6
HD = 64
EQ = 384
NQK = 768
DC = D // P      # 12
LT = L // P      # 16
ACH = 512        # attention lq chunk
XCH = 256        # qkv l chunk
F32 = mybir.dt.float32
F16 = mybir.dt.float16
AF = mybir.ActivationFunctionType


def build_bass(repeat=1):
    nc = bacc.Bacc("TRN2", target_bir_lowering=False, debug=False, num_devices=8)
    xT = nc.dram_tensor("xT", [D, L], F16, kind="ExternalInput")
    wqkT = nc.dram_tensor("wqkT", [D, NQK], F16, kind="ExternalInput")
    wvT = nc.dram_tensor("wvT", [D, EQ], F16, kind="ExternalInput")
    woT = nc.dram_tensor("woT", [EQ, D], F16, kind="ExternalInput")
    cos2 = nc.dram_tensor("cos2", [P, L], F16, kind="ExternalInput")
    ss2 = nc.dram_tensor("ss2", [P, L], F16, kind="ExternalInput")
    out = nc.dram_tensor("out", [L, D], F32, kind="ExternalOutput")

    xT_r = xT.rearrange("(dc p) l -> p dc l", p=P)
    wqkT_r = wqkT.rearrange("(dc p) e -> p dc e", p=P)
    wvT_r = wvT.rearrange("(dc p) e -> p dc e", p=P)
    woT_r = woT.rearrange("(ec p) d -> p ec d", p=P)

    with tile.TileContext(nc) as tc:
        rep_cm = tc.For_i(0, repeat, 1) if repeat > 1 else contextlib.nullcontext()
        with rep_cm, tc.tile_pool(name="persist", bufs=1) as persist:
            xsb = persist.tile([P, DC, L], F16)
            qT = persist.tile([P, 3, L], F16)
            kT = persist.tile([P, 3, L], F16)
            v1 = persist.tile([P, LT, HL, HD + 1], F16)
            cos_sb = persist.tile([P, L], F16)
            ss_sb = persist.tile([P, L], F16)
            outT = persist.tile([P, 3, L], F16)
            ones_c = nc.const_aps.tensor(1.0, (P, 1), F32)
            nc.vector.tensor_copy(
                v1[:, :, :, HD : HD + 1], ones_c.to_broadcast([P, LT, HL, 1])
            )
            nc.sync.dma_start(cos_sb[:], cos2[:])
            nc.sync.dma_start(ss_sb[:], ss2[:])
            for d0 in range(0, DC, 3):
                nc.sync.dma_start(
                    xsb[:, d0 : d0 + 3, :], xT_r[:, d0 : d0 + 3, :]
                )

            with (
                tc.tile_pool(name="s2w", bufs=2) as s2w,
                tc.tile_pool(name="s2t", bufs=2) as s2t,
                tc.tile_pool(name="s2att", bufs=2) as s2att,
                tc.tile_pool(name="s2o", bufs=3) as s2o,
                tc.tile_pool(name="s2nrm", bufs=3) as s2nrm,
                tc.tile_pool(name="ps_acc", bufs=2, space=bass.MemorySpace.PSUM) as ps_acc,
                tc.tile_pool(name="ps_s", bufs=2, space=bass.MemorySpace.PSUM) as ps_s,
                tc.tile_pool(name="ps_av", bufs=2, space=bass.MemorySpace.PSUM) as ps_av,
            ):
                for etp in range(3):
                    # --- load this pair's qk weights (and v weights at etp 0)
                    wqks = s2w.tile([P, DC, EQ], F16, tag="w")
                    for d0 in range(0, DC, 3):
                        dsl = slice(d0, d0 + 3)
                        nc.sync.dma_start(
                            wqks[:, dsl, 0:P], wqkT_r[:, dsl, etp * P : (etp + 1) * P]
                        )
                        nc.sync.dma_start(
                            wqks[:, dsl, P : 2 * P],
                            wqkT_r[:, dsl, EQ + etp * P : EQ + (etp + 1) * P],
                        )
                    if etp == 0:
                        wv_sb = s2w.tile([P, DC, EQ], F16, tag="w")
                        for d0 in range(0, DC, 3):
                            nc.sync.dma_start(
                                wv_sb[:, d0 : d0 + 3, :], wvT_r[:, d0 : d0 + 3, :]
                            )
                    if etp == 2:
                        wo_sb = s2w.tile([P, 3, D], F16, tag="w")
                        nc.sync.dma_start(wo_sb[:], woT_r[:])

                    # --- qk projection + rope (+ V on first pass); x resident
                    for c in range(L // XCH):
                        sl = slice(c * XCH, (c + 1) * XCH)

                        for half in range(2):  # 0: q tile, 1: k tile
                            ps = ps_acc.tile([P, ACH], F32, tag="acc")
                            for dc in range(DC):
                                nc.tensor.matmul(
                                    ps[:, 0:XCH],
                                    wqks[:, dc, half * P : (half + 1) * P],
                                    xsb[:, dc, sl],
                                    start=(dc == 0),
                                    stop=(dc == DC - 1),
                                )
                            dst = (qT if half == 0 else kT)[:, etp, sl]
                            tcos = s2t.tile([P, XCH], F32, tag="tcos")
                            trot = s2t.tile([P, XCH], F32, tag="trot")
                            nc.vector.tensor_mul(tcos[:], ps[:, 0:XCH], cos_sb[:, sl])
                            for q_ in range(4):
                                s = (q_ ^ 1) * 32
                                d_ = q_ * 32
                                nc.vector.tensor_mul(
                                    trot[d_ : d_ + 32, :],
                                    ps[s : s + 32, 0:XCH],
                                    ss_sb[d_ : d_ + 32, sl],
                                )
                            nc.vector.tensor_add(dst, tcos[:], trot[:])

                        if etp == 0:
                            for lt2 in range(XCH // P):
                                lk = c * (XCH // P) + lt2
                                pv = ps_acc.tile([P, ACH], F32, tag="acc")
                                for dc in range(DC):
                                    nc.tensor.matmul(
                                        pv[:, 0:EQ],
                                        xsb[:, dc, lk * P : (lk + 1) * P],
                                        wv_sb[:, dc, :],
                                        start=(dc == 0),
                                        stop=(dc == DC - 1),
                                    )
                                nc.scalar.copy(
                                    v1[:, lk, :, 0:HD],
                                    pv[:, 0:EQ].rearrange("p (h d) -> p h d", h=HL),
                                )

                    # --- attention for heads 2*etp (rows 0:64) and 2*etp+1
                    for cq in range(L // ACH):
                        cqs = slice(cq * ACH, (cq + 1) * ACH)
                        pav0 = ps_av.tile([HD + 1, ACH], F32, tag="av")
                        pav1 = ps_av.tile([HD + 1, ACH], F32, tag="av")
                        for lk in range(LT):
                            pscore = ps_s.tile([P, 2 * ACH], F32)
                            att = s2att.tile([P, 2 * ACH], F16)
                            for hh in range(2):  # row-tiled pair, concurrent
                                po = hh * HD
                                nc.tensor.matmul(
                                    pscore[:, hh * ACH : (hh + 1) * ACH],
                                    kT[po : po + HD, etp, lk * P : (lk + 1) * P],
                                    qT[po : po + HD, etp, cqs],
                                    start=True,
                                    stop=True,
                                )
                            nc.scalar.activation(att[:], pscore[:], AF.Exp, scale=0.125)
                            for hh, pav in ((0, pav0), (1, pav1)):
                                nc.tensor.matmul(
                                    pav[:],
                                    v1[:, lk, 2 * etp + hh, :],
                                    att[:, hh * ACH : (hh + 1) * ACH],
                                    start=(lk == 0),
                                    stop=(lk == LT - 1),
                                )
                        for hh, pav in ((0, pav0), (1, pav1)):
                            po = hh * HD
                            dcp = s2nrm.tile([1, ACH], F32, tag="dcp")
                            nc.vector.tensor_copy(dcp[:], pav[HD : HD + 1, :])
                            rcp = s2nrm.tile([1, ACH], F32, tag="rcp")
                            nc.vector.reciprocal_approx_fast(out=rcp[:], in_=dcp[:])
                            rb = s2nrm.tile([HD, ACH], F32, tag="rb")
                            nc.gpsimd.partition_broadcast(rb[:], rcp[:], channels=HD)
                            nc.vector.tensor_mul(
                                outT[po : po + HD, etp, cqs], pav[0:HD, :], rb[:]
                            )

                        if etp == 2:
                            # o-projection for this lq chunk (all heads done)
                            for lt in range(ACH // P):
                                l0 = cq * ACH + lt * P
                                for dn in range(D // ACH):
                                    pso = ps_acc.tile([P, ACH], F32, tag="acc")
                                    for ec in range(3):
                                        nc.tensor.matmul(
                                            pso[:],
                                            outT[:, ec, l0 : l0 + P],
                                            wo_sb[:, ec, dn * ACH : (dn + 1) * ACH],
                                            start=(ec == 0),
                                            stop=(ec == 2),
                                        )
                                    ot = s2o.tile([P, ACH], F32)
                                    nc.scalar.copy(ot[:], pso[:])
                                    nc.sync.dma_start(
                                        out[l0 : l0 + P, dn * ACH : (dn + 1) * ACH],
                                        ot[:],
                                    )

    nc.compile()
    return nc


_NC_CACHE = None


def _get_nc():
    global _NC_CACHE
    if _NC_CACHE is None:
        _NC_CACHE = build_bass()
    return _NC_CACHE


def make_in_maps(x, w_qkv, w_o, cos, sin):
    x = np.asarray(x, dtype=np.float32)
    w_qkv = np.asarray(w_qkv, dtype=np.float32)
    w_o = np.asarray(w_o, dtype=np.float32)
    cos = np.asarray(cos, dtype=np.float32)
    sin = np.asarray(sin, dtype=np.float32)

    cosT = np.ascontiguousarray(cos.T)
    sinT = sin.T
    ss = np.concatenate([-sinT[0:32], sinT[32:64]], axis=0)
    cos2 = np.ascontiguousarray(np.tile(cosT, (2, 1))).astype(np.float16)
    ss2 = np.ascontiguousarray(np.tile(ss, (2, 1))).astype(np.float16)

    in_maps = []
    for c in range(8):
        b, g = c // 4, c % 4
        xTc = np.ascontiguousarray(x[b].T).astype(np.float16)
        wq = w_qkv[g * EQ : (g + 1) * EQ]
        wk = w_qkv[D + g * EQ : D + (g + 1) * EQ]
        wv = w_qkv[2 * D + g * EQ : 2 * D + (g + 1) * EQ]
        wqkTc = np.ascontiguousarray(np.concatenate([wq, wk], 0).T).astype(np.float16)
        wvTc = np.ascontiguousarray(wv.T).astype(np.float16)
        woTc = np.ascontiguousarray(w_o[:, g * EQ : (g + 1) * EQ].T).astype(np.float16)
        in_maps.append(
            {
                "xT": xTc,
                "wqkT": wqkTc,
                "wvT": wvTc,
                "woT": woTc,
                "cos2": cos2,
                "ss2": ss2,
            }
        )
    return in_maps


def kernel(x, w_qkv, w_o, cos, sin):
    nc = _get_nc()
    in_maps = make_in_maps(x, w_qkv, w_o, cos, sin)
    res = run_bass_kernel_spmd(nc, in_maps, core_ids=list(range(8)))
    outs = [res.results[c]["out"] for c in range(8)]
    full = np.stack(
        [
            outs[0] + outs[1] + outs[2] + outs[3],
            outs[4] + outs[5] + outs[6] + outs[7],
        ]
    ).astype(np.float32)
    return full
